# revision 14
# baseline (speedup 1.0000x reference)
"""Trainium2 Bass kernel for nn_BinConv2d: BN(train-mode) -> sign -> 3x3 conv.

Two launches on 8 cores, batch-sharded (2 images/core, 128 partitions =
2 img x 64 ch):

  Launch A (stats), engine-split so neither engine is the wall: DVE
    bn_stats takes 66 of the 98 512-elem groups, ACT computes sum(x) /
    sum(x^2) for the other 32 via Copy/Square with accum_out (per-1024
    sub-groups to bound f32 accumulation error).  ACT chunks are placed
    early and a small DVE chunk last so the post-DMA tail is short.
    Host combines both shares in f64, pools across cores, and folds
    BN+sign into one per-channel threshold t_c = mean_c -
    bias_c*sqrt(var_c+eps)/w_c.

  Launch B (conv): per image pair, sign(x) runs 128 partitions wide
    (both images at once) on ACT into a tmp tile, then 4 SBUF->SBUF
    DMA copies (scalar queue for strip A, gpsimd for strip B) build the
    two per-image strips in fp8e4 ({-1,0,1} exact): partitions = 64 ch
    x 2 halves, second half shifted up one row-slot, so an AP
    strip[:, 2k*226+dx] yields rows 2k/2k+1 across the halves.
    Matmuls are double-tile: one instruction computes TWO 2-row tiles
    (moving AP [2, 224] with slot-pair stride), free size 448, psum
    tile [128, 2, 224] = one 2KB bank; 6 matmuls (2 row-pairs x 3 dx)
    accumulate a 4-row tile; 7 such tiles per image per 28-row band.
    Matmuls run weight-outer in half-band chunks (4+3 tiles) so two
    chunks share the 8 psum banks and evacuations (DVE tensor_scalar
    +bias, 448 wide) overlap the next chunk's matmuls.  y is written by
    gpsimd in the parity-split device layout [128, 2, 112, 224] and
    unshuffled on host.
"""

import sys

if "/opt/trn_rl_repo" not in sys.path:
    sys.path.insert(0, "/opt/trn_rl_repo")

import numpy as np

import concourse.bacc as bacc
import concourse.tile as tile
from concourse import mybir
from concourse.bass_utils import run_bass_kernel_spmd

F32 = mybir.dt.float32
F16 = mybir.dt.float16
F8 = mybir.dt.float8e4

N_CORES = 8
N, C, H, W = 16, 64, 224, 224
BN_EPS = 1e-4
BAND = 28              # output rows per band
NB = H // BAND         # 8 bands
WP = W + 2             # padded strip width (226)
NT = BAND // 2         # 14 2-row tiles per band
NU = BAND // 4         # 7 4-row (double) tiles per band
SLOTS = BAND + 2       # 30 strip slots per band
STRIP_LEN = SLOTS * WP
HH = H // 2            # 112


# stats chunking: (n_groups, engine); 'A' chunks go to ACT (sum/sum^2 via
# accum_out), 'D' chunks to DVE bn_stats.  ACT chunks early, small DVE
# chunk last to shorten the post-DMA tail.
STATS_CHUNKS = [(2, "D"), (13, "A"), (8, "D"), (13, "A"), (13, "D"),
                (6, "A"), (13, "D"), (13, "D"), (13, "D"), (4, "D")]
N_DVE_GROUPS = sum(g for g, e in STATS_CHUNKS if e == "D")  # 66
NAC = sum((g * 512 + 1023) // 1024 for g, e in STATS_CHUNKS if e == "A")  # 17


def build_stats_nc(repeat=1):
    """Per-core moments of x_s [128, 50176] f32, split across engines:
    DVE bn_stats for 66 of the 98 512-elem groups -> stats [128, 2]
    (mean, var over the DVE share); ACT computes per-1024-elem sums of x
    and x^2 via accum_out for the other 32 groups -> asum/asq [128, 17].
    The host combines both shares in f64."""
    nc = bacc.Bacc()
    cols = H * W
    x_s = nc.declare_dram_parameter("x_s", [128, cols], F32, isOutput=False)
    stats_out = nc.declare_dram_parameter("stats", [128, 2], F32, isOutput=True)
    asum_out = nc.declare_dram_parameter("asum", [128, NAC], F32, isOutput=True)
    asq_out = nc.declare_dram_parameter("asq", [128, NAC], F32, isOutput=True)

    assert sum(g for g, _ in STATS_CHUNKS) == cols // 512

    with tile.TileContext(nc) as tc:
        with (
            tc.tile_pool(name="xc", bufs=6) as xpool,
            tc.tile_pool(name="acc", bufs=1) as apool,
        ):
            stats = apool.tile([128, N_DVE_GROUPS, 6], F32)
            mv = apool.tile([128, 2], F32)
            asum = apool.tile([128, NAC], F32)
            asq = apool.tile([128, NAC], F32)
            trash = apool.tile([128, 1024], F16)

            def emit_all():
                g0 = 0
                di = 0
                ai = 0
                for ng, eng in STATS_CHUNKS:
                    xt = xpool.tile([128, 13 * 512], F32, tag="xt")
                    nc.sync.dma_start(
                        out=xt[:, : ng * 512],
                        in_=x_s[:, g0 * 512 : (g0 + ng) * 512],
                    )
                    if eng == "A":
                        off = 0
                        rem = ng * 512
                        while rem > 0:
                            sz = min(1024, rem)
                            nc.scalar.activation(
                                out=trash[:, 0:sz],
                                in_=xt[:, off : off + sz],
                                func=mybir.ActivationFunctionType.Copy,
                                accum_out=asum[:, ai : ai + 1],
                            )
                            nc.scalar.activation(
                                out=trash[:, 0:sz],
                                in_=xt[:, off : off + sz],
                                func=mybir.ActivationFunctionType.Square,
                                accum_out=asq[:, ai : ai + 1],
                            )
                            off += sz
                            rem -= sz
                            ai += 1
                    else:
                        for g in range(ng):
                            nc.vector.bn_stats(
                                out=stats[:, di, :],
                                in_=xt[:, g * 512 : (g + 1) * 512],
                            )
                            di += 1
                    g0 += ng
                assert ai == NAC and di == N_DVE_GROUPS
                nc.vector.bn_aggr(out=mv[:], in_=stats[:])
                nc.sync.dma_start(out=stats_out[:], in_=mv[:])
                nc.scalar.dma_start(out=asum_out[:], in_=asum[:])
                nc.scalar.dma_start(out=asq_out[:], in_=asq[:])

            if repeat == 1:
                emit_all()
            else:
                with tc.For_i(0, repeat, 1):
                    emit_all()
    nc.compile()
    return nc


def build_conv_nc(repeat=1):
    """Per-core conv kernel: x_b [128, 224, 224] f32 (2 img x 64 ch),
    wts [128, 12, 128] fp16 lhsT bank, tneg [128,1], cbias [128,1]
    -> y [128, 2, 112, 224] f32 (parity-split device layout)."""
    nc = bacc.Bacc()
    x_b = nc.declare_dram_parameter("x_b", [128, H, W], F32, isOutput=False)
    wts = nc.declare_dram_parameter("wts", [128, 12, 128], F16, isOutput=False)
    tneg = nc.declare_dram_parameter("tneg", [128, 1], F32, isOutput=False)
    cbias = nc.declare_dram_parameter("cbias", [128, 1], F32, isOutput=False)
    y = nc.declare_dram_parameter("y", [128, 2, HH, W], F16, isOutput=True)

    with tile.TileContext(nc) as tc:
        with (
            tc.tile_pool(name="const", bufs=1) as cpool,
            tc.tile_pool(name="xband", bufs=3) as xpool,
            tc.tile_pool(name="stage", bufs=2) as opool,
            tc.tile_pool(name="psum", bufs=8, space="PSUM") as ppool,
        ):
            wsb = cpool.tile([128, 12, 128], F16)
            nc.sync.dma_start(out=wsb[:], in_=wts[:])
            tsb = cpool.tile([128, 1], F32)
            nc.sync.dma_start(out=tsb[:], in_=tneg[:])
            bsb = cpool.tile([128, 1], F32)
            nc.sync.dma_start(out=bsb[:], in_=cbias[:])

            strips = [
                [
                    cpool.tile([128, STRIP_LEN], F8, name=f"strip{im}_{pb}",
                               tag=f"strip{im}_{pb}")
                    for pb in range(2)
                ]
                for im in range(2)
            ]
            for im in range(2):
                for pb in range(2):
                    s3 = strips[im][pb].rearrange("p (s c) -> p s c", c=WP)
                    nc.vector.memset(s3[:, :, 0], 0.0)
                    nc.vector.memset(s3[:, :, WP - 1], 0.0)
                    nc.vector.memset(s3[:, 0, :], 0.0)
                    nc.vector.memset(s3[:, SLOTS - 1, :], 0.0)

            # 3 manually-rotated sign buffers; pad cols zeroed once so the
            # strip copies can move whole slots as one contiguous range
            # per partition (2D-strided copies cost ~4x more dispatch).
            tmps = [cpool.tile([128, 12, WP], F8, name=f"tmp{i}")
                    for i in range(3)]
            for t3 in tmps:
                nc.vector.memset(t3[:, :, 0], 0.0)
                nc.vector.memset(t3[:, :, WP - 1], 0.0)

            def load_band(b, queue=None):
                # split loads: ACT chunks 1+2 (slots <18) can start after
                # the first piece, and smaller transfers reduce DMA-engine
                # head-of-line blocking of the latency-critical strip
                # copies (the engines are shared by all queues).
                r0 = b * BAND
                lo_r = max(r0 - 1, 0)
                hi_r = min(r0 + BAND + 1, H)
                s0 = lo_r - (r0 - 1)
                nr = hi_r - lo_r
                q = queue or nc.sync
                xt = xpool.tile([128, SLOTS, W], F32, tag="xt")
                q.dma_start(out=xt[:, s0:18, :],
                            in_=x_b[:, lo_r : lo_r + 18 - s0, :])
                q.dma_start(out=xt[:, 18 : s0 + nr, :],
                            in_=x_b[:, lo_r + 18 - s0 : hi_r, :])
                return xt, s0, nr

            def sign_band(b, xt, s0, nr):
                # binarize 128 wide (both images at once) into tmp, then
                # 4 DMA copies build the strips: direct halves at slots
                # [lo,hi), shifted halves at [lo-1, hi-1).  Runs one band
                # ahead of the matmuls, so copy latency (DMA-engine
                # contention with the bulk x/y traffic) is off the
                # critical path.
                sA = strips[0][b % 2]
                sB = strips[1][b % 2]
                s3A = sA.rearrange("p (s c) -> p s c", c=WP)
                s3B = sB.rearrange("p (s c) -> p s c", c=WP)

                if b == NB - 1:
                    # shifted halves' bottom pad: slot 28 holds the
                    # (zero) slot-29 data; stale from band NB-3.
                    nc.vector.memset(s3A[64:128, SLOTS - 2, :], 0.0)
                    nc.vector.memset(s3B[0:64, SLOTS - 2, :], 0.0)

                chunks = ((s0, 10), (10, 18), (18, s0 + nr))
                for ci, (lo, hi) in enumerate(chunks):
                    ns = hi - lo
                    tmp = tmps[ci]
                    nc.scalar.activation(
                        out=tmp[:, 0:ns, 1 : 1 + W],
                        in_=xt[:, lo:hi, :],
                        func=mybir.ActivationFunctionType.Sign,
                        bias=tsb[:],
                    )
                    tlo = max(lo - 1, 0)
                    j0 = tlo - lo + 1
                    nc.scalar.dma_start(
                        out=s3A[0:64, lo:hi, :],
                        in_=tmp[0:64, 0:ns, :],
                    )
                    nc.scalar.dma_start(
                        out=s3A[64:128, tlo : hi - 1, :],
                        in_=tmp[0:64, j0:ns, :],
                    )
                    nc.gpsimd.dma_start(
                        out=s3B[64:128, lo:hi, :],
                        in_=tmp[64:128, 0:ns, :],
                    )
                    nc.gpsimd.dma_start(
                        out=s3B[0:64, tlo : hi - 1, :],
                        in_=tmp[64:128, j0:ns, :],
                    )

            def matmul_band(b):
                r0 = b * BAND
                g4A = strips[0][b % 2].rearrange("p (g t c) -> p g t c",
                                                 t=2, c=WP)
                g4B = strips[1][b % 2].rearrange("p (g t c) -> p g t c",
                                                 t=2, c=WP)
                stgs = {}
                QT = ((0, 4), (4, NU))

                def do_chunk(im, q):
                    g4 = g4A if im == 0 else g4B
                    if im not in stgs:
                        stgs[im] = opool.tile([128, NT, W], F16,
                                              tag=f"stg{im}",
                                              name=f"stg{b}_{im}")
                    stg = stgs[im]
                    ua, ub = QT[q]
                    # weight-outer per half-band: 3-4 live psum banks,
                    # so two chunks fit in the 8 banks and the next
                    # chunk's matmuls overlap this one's evacs.
                    pss = [
                        ppool.tile([128, 2, W], F32, tag="ps",
                                   name=f"ps{b}_{im}_{u}")
                        for u in range(ua, ub)
                    ]
                    for m in range(6):
                        pair, dx = divmod(m, 3)
                        for j, u in enumerate(range(ua, ub)):
                            g = 2 * u + pair  # slot 4u+2*pair, even
                            nc.tensor.matmul(
                                pss[j][:, :, :],
                                wsb[:, im * 6 + m, :],
                                g4[:, g : g + 2, 0, dx : dx + W],
                                start=(m == 0),
                                stop=(m == 5),
                            )
                    for j, u in enumerate(range(ua, ub)):
                        nc.vector.tensor_scalar(
                            out=stg[:, 2 * u : 2 * u + 2, :],
                            in0=pss[j][:, :, :],
                            scalar1=bsb[:],
                            scalar2=None,
                            op0=mybir.AluOpType.add,
                        )
                    h0 = 0 if q == 0 else NT // 2
                    # last band: write y from the sync queue (its xt loads
                    # are long done) so the gpsimd drain doesn't wait on
                    # the final y transfers.
                    yq = nc.sync if b == NB - 1 else nc.gpsimd
                    yq.dma_start(
                        out=y[:, im,
                              r0 // 2 + h0 : r0 // 2 + h0 + NT // 2, :],
                        in_=stg[:, h0 : h0 + NT // 2, :],
                    )

                for im, q in ((0, 0), (1, 0), (0, 1), (1, 1)):
                    do_chunk(im, q)

            def emit_all():
                # software pipeline: xt(b+2) loads | sign+copies(b+1) |
                # matmuls(b).  Band 1's load goes on the scalar queue so
                # its bulk transfer starts only after band 0's strip
                # copies are dispatched (otherwise it starves them and
                # delays the first matmul by ~12us).
                xts = {0: load_band(0)}
                sign_band(0, *xts.pop(0))
                xts[1] = load_band(1, queue=nc.scalar)
                for b in range(NB):
                    if b + 2 < NB:
                        xts[b + 2] = load_band(b + 2)
                    if b + 1 < NB:
                        sign_band(b + 1, *xts.pop(b + 1))
                    matmul_band(b)

            if repeat == 1:
                emit_all()
            else:
                with tc.For_i(0, repeat, 1):
                    emit_all()
    nc.compile()
    return nc


_cache = {}


def _get(name, builder):
    if name not in _cache:
        _cache[name] = builder()
    return _cache[name]


def _prep_conv_inputs(x, bn_weight, bn_bias, conv_weight, conv_bias, stats):
    # per-core results: DVE share (mean, var over 66*512 elems) + ACT
    # share (17 sums of x and x^2) -> exact per-(img,ch) moments in f64,
    # then pool to per-channel batch stats
    ipc = N // N_CORES
    cols = H * W
    n_dve = N_DVE_GROUPS * 512
    meanM = np.empty((N, C), np.float64)
    varM = np.empty((N, C), np.float64)
    for c in range(N_CORES):
        r = stats[c]
        mv = r["stats"].astype(np.float64)
        tot = mv[:, 0] * n_dve + r["asum"].astype(np.float64).sum(1)
        tot2 = (mv[:, 1] + mv[:, 0] ** 2) * n_dve + r["asq"].astype(
            np.float64
        ).sum(1)
        mean_p = tot / cols
        var_p = tot2 / cols - mean_p**2
        s = np.stack([mean_p, var_p], -1).reshape(ipc, C, 2)
        meanM[ipc * c : ipc * (c + 1)] = s[..., 0]
        varM[ipc * c : ipc * (c + 1)] = s[..., 1]
    m = meanM.mean(axis=0)
    v = (varM + meanM**2).mean(axis=0) - m**2
    t = m - bn_bias.astype(np.float64) * np.sqrt(v + BN_EPS) / bn_weight.astype(
        np.float64
    )
    tneg = np.tile((-t).astype(np.float32), 2)[:, None]  # [128,1]
    cb = np.tile(conv_bias.astype(np.float32), 2)[:, None]

    # lhsT bank [128, 12, 128]: m = img*6 + pair*3 + dx.
    wts = np.zeros((128, 12, 128), np.float32)
    for im in range(2):
        for pair in range(2):
            for dx in range(3):
                mi = im * 6 + pair * 3 + dx
                for h in range(2):
                    a_slot = h if im == 0 else 1 - h
                    for bcol in range(2):
                        dy = a_slot - bcol + 2 * pair
                        if 0 <= dy <= 2:
                            wts[
                                h * 64 : h * 64 + 64,
                                mi,
                                bcol * 64 : bcol * 64 + 64,
                            ] = conv_weight[:, :, dy, dx].T
    return wts.astype(np.float16), tneg, cb


def _unshuffle_y(arr, ipc):
    # arr [128, 2, 112, 224] f16: [b*64+oc, im, r2, col] -> [im, oc, 2*r2+b, col]
    a = arr.astype(np.float32).reshape(2, C, 2, HH, W)  # [b, oc, im, r2, col]
    a = a.transpose(2, 1, 3, 0, 4)             # [im, oc, r2, b, col]
    return a.reshape(ipc, C, H, W)


def kernel(x, bn_weight, bn_bias, conv_weight, conv_bias):
    x = np.ascontiguousarray(np.asarray(x), dtype=np.float32)
    bn_weight = np.asarray(bn_weight, dtype=np.float32)
    bn_bias = np.asarray(bn_bias, dtype=np.float32)
    conv_weight = np.asarray(conv_weight, dtype=np.float32)
    conv_bias = np.asarray(conv_bias, dtype=np.float32)

    ipc = N // N_CORES
    nc_s = _get("stats", build_stats_nc)
    in_maps = [
        {"x_s": x[ipc * c : ipc * (c + 1)].reshape(128, H * W)}
        for c in range(N_CORES)
    ]
    res = run_bass_kernel_spmd(nc_s, in_maps, list(range(N_CORES))).results
    stats = [res[c] for c in range(N_CORES)]

    wts, tneg, cb = _prep_conv_inputs(
        x, bn_weight, bn_bias, conv_weight, conv_bias, stats
    )

    nc_c = _get("conv", build_conv_nc)
    in_maps = [
        {
            "x_b": x[ipc * c : ipc * (c + 1)].reshape(128, H, W),
            "wts": wts,
            "tneg": tneg,
            "cbias": cb,
        }
        for c in range(N_CORES)
    ]
    res = run_bass_kernel_spmd(nc_c, in_maps, list(range(N_CORES))).results
    y = np.concatenate(
        [_unshuffle_y(res[c]["y"], ipc) for c in range(N_CORES)], axis=0
    )
    return y


# revision 17
# speedup vs baseline: 1.0187x; 1.0187x over previous
"""Trainium2 Bass kernel for nn_BinConv2d: BN(train-mode) -> sign -> 3x3 conv.

Two launches on 8 cores, batch-sharded (2 images/core, 128 partitions =
2 img x 64 ch):

  Launch A (stats), engine-split so neither engine is the wall: DVE
    bn_stats takes 66 of the 98 512-elem groups, ACT computes sum(x) /
    sum(x^2) for the other 32 via Copy/Square with accum_out (per-1024
    sub-groups to bound f32 accumulation error).  ACT chunks are placed
    early and a small DVE chunk last so the post-DMA tail is short.
    Host combines both shares in f64, pools across cores, and folds
    BN+sign into one per-channel threshold t_c = mean_c -
    bias_c*sqrt(var_c+eps)/w_c.

  Launch B (conv): per image pair, sign(x) runs 128 partitions wide
    (both images at once) on ACT into a tmp tile, then 4 SBUF->SBUF
    DMA copies (scalar queue for strip A, gpsimd for strip B) build the
    two per-image strips in fp8e4 ({-1,0,1} exact): partitions = 64 ch
    x 2 halves, second half shifted up one row-slot, so an AP
    strip[:, 2k*226+dx] yields rows 2k/2k+1 across the halves.
    Matmuls are double-tile: one instruction computes TWO 2-row tiles
    (moving AP [2, 224] with slot-pair stride), free size 448, psum
    tile [128, 2, 224] = one 2KB bank; 6 matmuls (2 row-pairs x 3 dx)
    accumulate a 4-row tile; 7 such tiles per image per 28-row band.
    Matmuls run weight-outer in half-band chunks (4+3 tiles) so two
    chunks share the 8 psum banks and evacuations (DVE tensor_scalar
    +bias, 448 wide) overlap the next chunk's matmuls.  y is written by
    gpsimd in the parity-split device layout [128, 2, 112, 224] and
    unshuffled on host.
"""

import sys

if "/opt/trn_rl_repo" not in sys.path:
    sys.path.insert(0, "/opt/trn_rl_repo")

import numpy as np

import concourse.bacc as bacc
import concourse.tile as tile
from concourse import mybir
from concourse.bass_utils import run_bass_kernel_spmd

F32 = mybir.dt.float32
F16 = mybir.dt.float16
F8 = mybir.dt.float8e4

N_CORES = 8
N, C, H, W = 16, 64, 224, 224
BN_EPS = 1e-4
BAND = 28              # output rows per band
NB = H // BAND         # 8 bands
WP = W + 2             # padded strip width (226)
NT = BAND // 2         # 14 2-row tiles per band
NU = BAND // 4         # 7 4-row (double) tiles per band
SLOTS = BAND + 2       # 30 strip slots per band
STRIP_LEN = SLOTS * WP
HH = H // 2            # 112


# stats chunking: (n_groups, engine); 'A' chunks go to ACT (sum/sum^2 via
# accum_out), 'D' chunks to DVE bn_stats.  ACT chunks early, small DVE
# chunk last to shorten the post-DMA tail.
STATS_CHUNKS = [(2, "D"), (13, "A"), (8, "D"), (13, "A"), (13, "D"),
                (6, "A"), (13, "D"), (13, "D"), (13, "D"), (4, "D")]
N_DVE_GROUPS = sum(g for g, e in STATS_CHUNKS if e == "D")  # 66
NAC = sum((g * 512 + 1023) // 1024 for g, e in STATS_CHUNKS if e == "A")  # 17


def build_stats_nc(repeat=1):
    """Per-core moments of x_s [128, 50176] f32, split across engines:
    DVE bn_stats for 66 of the 98 512-elem groups -> stats [128, 2]
    (mean, var over the DVE share); ACT computes per-1024-elem sums of x
    and x^2 via accum_out for the other 32 groups -> asum/asq [128, 17].
    The host combines both shares in f64."""
    nc = bacc.Bacc()
    cols = H * W
    x_s = nc.declare_dram_parameter("x_s", [128, cols], F32, isOutput=False)
    stats_out = nc.declare_dram_parameter("stats", [128, 2], F32, isOutput=True)
    asum_out = nc.declare_dram_parameter("asum", [128, NAC], F32, isOutput=True)
    asq_out = nc.declare_dram_parameter("asq", [128, NAC], F32, isOutput=True)

    assert sum(g for g, _ in STATS_CHUNKS) == cols // 512

    with tile.TileContext(nc) as tc:
        with (
            tc.tile_pool(name="xc", bufs=6) as xpool,
            tc.tile_pool(name="acc", bufs=1) as apool,
        ):
            stats = apool.tile([128, N_DVE_GROUPS, 6], F32)
            mv = apool.tile([128, 2], F32)
            asum = apool.tile([128, NAC], F32)
            asq = apool.tile([128, NAC], F32)
            trash = apool.tile([128, 1024], F16)

            def emit_all():
                g0 = 0
                di = 0
                ai = 0
                for ng, eng in STATS_CHUNKS:
                    xt = xpool.tile([128, 13 * 512], F32, tag="xt")
                    nc.sync.dma_start(
                        out=xt[:, : ng * 512],
                        in_=x_s[:, g0 * 512 : (g0 + ng) * 512],
                    )
                    if eng == "A":
                        off = 0
                        rem = ng * 512
                        while rem > 0:
                            sz = min(1024, rem)
                            nc.scalar.activation(
                                out=trash[:, 0:sz],
                                in_=xt[:, off : off + sz],
                                func=mybir.ActivationFunctionType.Copy,
                                accum_out=asum[:, ai : ai + 1],
                            )
                            nc.scalar.activation(
                                out=trash[:, 0:sz],
                                in_=xt[:, off : off + sz],
                                func=mybir.ActivationFunctionType.Square,
                                accum_out=asq[:, ai : ai + 1],
                            )
                            off += sz
                            rem -= sz
                            ai += 1
                    else:
                        for g in range(ng):
                            nc.vector.bn_stats(
                                out=stats[:, di, :],
                                in_=xt[:, g * 512 : (g + 1) * 512],
                            )
                            di += 1
                    g0 += ng
                assert ai == NAC and di == N_DVE_GROUPS
                nc.vector.bn_aggr(out=mv[:], in_=stats[:])
                nc.sync.dma_start(out=stats_out[:], in_=mv[:])
                nc.scalar.dma_start(out=asum_out[:], in_=asum[:])
                nc.scalar.dma_start(out=asq_out[:], in_=asq[:])

            if repeat == 1:
                emit_all()
            else:
                with tc.For_i(0, repeat, 1):
                    emit_all()
    nc.compile()
    return nc


def build_conv_nc(repeat=1):
    """Per-core conv kernel: x_b [128, 224, 224] f32 (2 img x 64 ch),
    wts [128, 12, 128] fp16 lhsT bank, tneg [128,1], cbias [128,1]
    -> y [128, 2, 112, 224] f32 (parity-split device layout)."""
    nc = bacc.Bacc()
    x_b = nc.declare_dram_parameter("x_b", [128, H, W], F32, isOutput=False)
    wts = nc.declare_dram_parameter("wts", [128, 12, 128], F16, isOutput=False)
    tneg = nc.declare_dram_parameter("tneg", [128, 1], F32, isOutput=False)
    cbias = nc.declare_dram_parameter("cbias", [128, 1], F32, isOutput=False)
    y = nc.declare_dram_parameter("y", [128, 2, HH, W], F16, isOutput=True)

    with tile.TileContext(nc) as tc:
        with (
            tc.tile_pool(name="const", bufs=1) as cpool,
            tc.tile_pool(name="xband", bufs=2) as xpool,
            tc.tile_pool(name="stage", bufs=2) as opool,
            tc.tile_pool(name="psum", bufs=8, space="PSUM") as ppool,
        ):
            wsb = cpool.tile([128, 12, 128], F16)
            nc.sync.dma_start(out=wsb[:], in_=wts[:])
            tsb = cpool.tile([128, 1], F32)
            nc.sync.dma_start(out=tsb[:], in_=tneg[:])
            bsb = cpool.tile([128, 1], F32)
            nc.sync.dma_start(out=bsb[:], in_=cbias[:])

            strips = [
                [
                    cpool.tile([128, STRIP_LEN], F8, name=f"strip{im}_{pb}",
                               tag=f"strip{im}_{pb}")
                    for pb in range(2)
                ]
                for im in range(2)
            ]
            for im in range(2):
                for pb in range(2):
                    s3 = strips[im][pb].rearrange("p (s c) -> p s c", c=WP)
                    nc.vector.memset(s3[:, :, 0], 0.0)
                    nc.vector.memset(s3[:, :, WP - 1], 0.0)
                    nc.vector.memset(s3[:, 0, :], 0.0)
                    nc.vector.memset(s3[:, SLOTS - 1, :], 0.0)

            # 3 manually-rotated sign buffers; pad cols zeroed once so the
            # strip copies can move whole slots as one contiguous range
            # per partition (2D-strided copies cost ~4x more dispatch).
            tmps = [cpool.tile([128, 12, WP], F8, name=f"tmp{i}")
                    for i in range(3)]
            for t3 in tmps:
                nc.vector.memset(t3[:, :, 0], 0.0)
                nc.vector.memset(t3[:, :, WP - 1], 0.0)

            def load_band(b):
                # All x loads go on the sync queue: packets are FIFO
                # within one DMA queue, so earlier (more urgent) pieces
                # complete first; separate queues would race for the
                # shared DMA engines.  3 pieces aligned with the sign
                # chunks let ACT start after the first ~1.1MB.  With
                # xpool bufs=2, band b+2's load waits on sign(b)'s
                # buffer release -- natural pacing one band ahead.
                r0 = b * BAND
                lo_r = max(r0 - 1, 0)
                hi_r = min(r0 + BAND + 1, H)
                s0 = lo_r - (r0 - 1)
                nr = hi_r - lo_r
                xt = xpool.tile([128, SLOTS, W], F32, tag="xt")
                for slo, shi in ((s0, 10), (10, 18), (18, s0 + nr)):
                    nc.sync.dma_start(
                        out=xt[:, slo:shi, :],
                        in_=x_b[:, r0 - 1 + slo : r0 - 1 + shi, :])
                return xt, s0, nr

            def sign_band(b, xt, s0, nr):
                # binarize 128 wide (both images at once) into tmp, then
                # 4 DMA copies build the strips: direct halves at slots
                # [lo,hi), shifted halves at [lo-1, hi-1).  Runs one band
                # ahead of the matmuls, so copy latency (DMA-engine
                # contention with the bulk x/y traffic) is off the
                # critical path.
                sA = strips[0][b % 2]
                sB = strips[1][b % 2]
                s3A = sA.rearrange("p (s c) -> p s c", c=WP)
                s3B = sB.rearrange("p (s c) -> p s c", c=WP)

                if b == NB - 1:
                    # shifted halves' bottom pad: slot 28 holds the
                    # (zero) slot-29 data; stale from band NB-3.
                    nc.vector.memset(s3A[64:128, SLOTS - 2, :], 0.0)
                    nc.vector.memset(s3B[0:64, SLOTS - 2, :], 0.0)

                chunks = ((s0, 10), (10, 18), (18, s0 + nr))
                for ci, (lo, hi) in enumerate(chunks):
                    ns = hi - lo
                    tmp = tmps[ci]
                    nc.scalar.activation(
                        out=tmp[:, 0:ns, 1 : 1 + W],
                        in_=xt[:, lo:hi, :],
                        func=mybir.ActivationFunctionType.Sign,
                        bias=tsb[:],
                    )
                    tlo = max(lo - 1, 0)
                    j0 = tlo - lo + 1
                    nc.scalar.dma_start(
                        out=s3A[0:64, lo:hi, :],
                        in_=tmp[0:64, 0:ns, :],
                    )
                    nc.scalar.dma_start(
                        out=s3A[64:128, tlo : hi - 1, :],
                        in_=tmp[0:64, j0:ns, :],
                    )
                    nc.gpsimd.dma_start(
                        out=s3B[64:128, lo:hi, :],
                        in_=tmp[64:128, 0:ns, :],
                    )
                    nc.gpsimd.dma_start(
                        out=s3B[0:64, tlo : hi - 1, :],
                        in_=tmp[64:128, j0:ns, :],
                    )

            def matmul_band(b):
                r0 = b * BAND
                g4A = strips[0][b % 2].rearrange("p (g t c) -> p g t c",
                                                 t=2, c=WP)
                g4B = strips[1][b % 2].rearrange("p (g t c) -> p g t c",
                                                 t=2, c=WP)
                stgs = {}
                QT = ((0, 4), (4, NU))

                def do_chunk(im, q):
                    g4 = g4A if im == 0 else g4B
                    if im not in stgs:
                        stgs[im] = opool.tile([128, NT, W], F16,
                                              tag=f"stg{im}",
                                              name=f"stg{b}_{im}")
                    stg = stgs[im]
                    ua, ub = QT[q]
                    # weight-outer per half-band: 3-4 live psum banks,
                    # so two chunks fit in the 8 banks and the next
                    # chunk's matmuls overlap this one's evacs.
                    pss = [
                        ppool.tile([128, 2, W], F32, tag="ps",
                                   name=f"ps{b}_{im}_{u}")
                        for u in range(ua, ub)
                    ]
                    for m in range(6):
                        pair, dx = divmod(m, 3)
                        for j, u in enumerate(range(ua, ub)):
                            g = 2 * u + pair  # slot 4u+2*pair, even
                            nc.tensor.matmul(
                                pss[j][:, :, :],
                                wsb[:, im * 6 + m, :],
                                g4[:, g : g + 2, 0, dx : dx + W],
                                start=(m == 0),
                                stop=(m == 5),
                            )
                    for j, u in enumerate(range(ua, ub)):
                        nc.vector.tensor_scalar(
                            out=stg[:, 2 * u : 2 * u + 2, :],
                            in0=pss[j][:, :, :],
                            scalar1=bsb[:],
                            scalar2=None,
                            op0=mybir.AluOpType.add,
                        )
                    h0 = 0 if q == 0 else NT // 2
                    # last band: write y from the sync queue (its xt loads
                    # are long done) so the gpsimd drain doesn't wait on
                    # the final y transfers.
                    yq = nc.sync if b == NB - 1 else nc.gpsimd
                    yq.dma_start(
                        out=y[:, im,
                              r0 // 2 + h0 : r0 // 2 + h0 + NT // 2, :],
                        in_=stg[:, h0 : h0 + NT // 2, :],
                    )

                for im, q in ((0, 0), (1, 0), (0, 1), (1, 1)):
                    do_chunk(im, q)

            def emit_all():
                # software pipeline: xt(b+2) loads | sign+copies(b+1) |
                # matmuls(b)
                xts = {0: load_band(0)}
                sign_band(0, *xts.pop(0))
                xts[1] = load_band(1)
                for b in range(NB):
                    if b + 2 < NB:
                        xts[b + 2] = load_band(b + 2)
                    if b + 1 < NB:
                        sign_band(b + 1, *xts.pop(b + 1))
                    matmul_band(b)

            if repeat == 1:
                emit_all()
            else:
                with tc.For_i(0, repeat, 1):
                    emit_all()
    nc.compile()
    return nc


_cache = {}


def _get(name, builder):
    if name not in _cache:
        _cache[name] = builder()
    return _cache[name]


def _prep_conv_inputs(x, bn_weight, bn_bias, conv_weight, conv_bias, stats):
    # per-core results: DVE share (mean, var over 66*512 elems) + ACT
    # share (17 sums of x and x^2) -> exact per-(img,ch) moments in f64,
    # then pool to per-channel batch stats
    ipc = N // N_CORES
    cols = H * W
    n_dve = N_DVE_GROUPS * 512
    meanM = np.empty((N, C), np.float64)
    varM = np.empty((N, C), np.float64)
    for c in range(N_CORES):
        r = stats[c]
        mv = r["stats"].astype(np.float64)
        tot = mv[:, 0] * n_dve + r["asum"].astype(np.float64).sum(1)
        tot2 = (mv[:, 1] + mv[:, 0] ** 2) * n_dve + r["asq"].astype(
            np.float64
        ).sum(1)
        mean_p = tot / cols
        var_p = tot2 / cols - mean_p**2
        s = np.stack([mean_p, var_p], -1).reshape(ipc, C, 2)
        meanM[ipc * c : ipc * (c + 1)] = s[..., 0]
        varM[ipc * c : ipc * (c + 1)] = s[..., 1]
    m = meanM.mean(axis=0)
    v = (varM + meanM**2).mean(axis=0) - m**2
    t = m - bn_bias.astype(np.float64) * np.sqrt(v + BN_EPS) / bn_weight.astype(
        np.float64
    )
    tneg = np.tile((-t).astype(np.float32), 2)[:, None]  # [128,1]
    cb = np.tile(conv_bias.astype(np.float32), 2)[:, None]

    # lhsT bank [128, 12, 128]: m = img*6 + pair*3 + dx.
    wts = np.zeros((128, 12, 128), np.float32)
    for im in range(2):
        for pair in range(2):
            for dx in range(3):
                mi = im * 6 + pair * 3 + dx
                for h in range(2):
                    a_slot = h if im == 0 else 1 - h
                    for bcol in range(2):
                        dy = a_slot - bcol + 2 * pair
                        if 0 <= dy <= 2:
                            wts[
                                h * 64 : h * 64 + 64,
                                mi,
                                bcol * 64 : bcol * 64 + 64,
                            ] = conv_weight[:, :, dy, dx].T
    return wts.astype(np.float16), tneg, cb


def _unshuffle_y(arr, ipc):
    # arr [128, 2, 112, 224] f16: [b*64+oc, im, r2, col] -> [im, oc, 2*r2+b, col]
    a = arr.astype(np.float32).reshape(2, C, 2, HH, W)  # [b, oc, im, r2, col]
    a = a.transpose(2, 1, 3, 0, 4)             # [im, oc, r2, b, col]
    return a.reshape(ipc, C, H, W)


def kernel(x, bn_weight, bn_bias, conv_weight, conv_bias):
    x = np.ascontiguousarray(np.asarray(x), dtype=np.float32)
    bn_weight = np.asarray(bn_weight, dtype=np.float32)
    bn_bias = np.asarray(bn_bias, dtype=np.float32)
    conv_weight = np.asarray(conv_weight, dtype=np.float32)
    conv_bias = np.asarray(conv_bias, dtype=np.float32)

    ipc = N // N_CORES
    nc_s = _get("stats", build_stats_nc)
    in_maps = [
        {"x_s": x[ipc * c : ipc * (c + 1)].reshape(128, H * W)}
        for c in range(N_CORES)
    ]
    res = run_bass_kernel_spmd(nc_s, in_maps, list(range(N_CORES))).results
    stats = [res[c] for c in range(N_CORES)]

    wts, tneg, cb = _prep_conv_inputs(
        x, bn_weight, bn_bias, conv_weight, conv_bias, stats
    )

    nc_c = _get("conv", build_conv_nc)
    in_maps = [
        {
            "x_b": x[ipc * c : ipc * (c + 1)].reshape(128, H, W),
            "wts": wts,
            "tneg": tneg,
            "cbias": cb,
        }
        for c in range(N_CORES)
    ]
    res = run_bass_kernel_spmd(nc_c, in_maps, list(range(N_CORES))).results
    y = np.concatenate(
        [_unshuffle_y(res[c]["y"], ipc) for c in range(N_CORES)], axis=0
    )
    return y
